# revision 1
# baseline (speedup 1.0000x reference)
"""H2GCN neighborhood aggregation on 8 Trainium2 NeuronCores.

Computes concat([adj_t @ x, adj_t2 @ x], axis=1) for
adj_t/adj_t2: [8192, 8192] f32, x: [8192, 256] f32.

Sharding: row-shard adj_t/adj_t2 (1024 rows per core), replicate x,
each core produces its [1024, 512] slice of the output.

Per-core dataflow (memory-bound, ~74 MB HBM traffic per core):
  - x loaded once to SBUF (optionally cast f32->bf16 in the DMA).
  - A-row-block tiles streamed in naturally ([128, 2048] chunks),
    PE-transposed 128x128 blocks -> PSUM -> copied to SBUF (DVE/ACT
    alternating), then used as the stationary operand of bf16/f32r
    matmuls accumulating over k into PSUM [128, 256].
"""

import numpy as np

N = 8192
D = 256
CORES = 8
P = 128
M_LOC = N // CORES  # 1024 rows of each adjacency matrix per core
MB = M_LOC // P  # 8 output row-blocks per core
KB = N // P  # 64 contraction blocks
KB_PER_CHUNK = 16  # A streamed in [128, 2048] chunks
N_CHUNKS = KB // KB_PER_CHUNK  # 4
GRP = 8  # k-blocks per transpose/copy group (one PSUM bank)
N_GRP = KB // GRP  # 8

MODE = "bf16"  # "bf16" or "f32r"

_cache = {}


def _build(mode):
    import concourse.bacc as bacc
    import concourse.tile as tile
    import concourse.mybir as mybir

    F32 = mybir.dt.float32
    if mode == "bf16":
        CDT = mybir.dt.bfloat16
        in_dt = F32
        load_engine = None  # gpsimd (cast in DMA)
    else:
        CDT = mybir.dt.float32r
        in_dt = mybir.dt.float32r
        load_engine = "sync"

    nc = bacc.Bacc(
        "TRN2",
        target_bir_lowering=False,
        debug=False,
        enable_asserts=False,
        num_devices=CORES,
    )
    a_ap = nc.dram_tensor("a", [M_LOC, N], in_dt, kind="ExternalInput").ap()
    a2_ap = nc.dram_tensor("a2", [M_LOC, N], in_dt, kind="ExternalInput").ap()
    x_ap = nc.dram_tensor("x", [N, D], in_dt, kind="ExternalInput").ap()
    id_ap = nc.dram_tensor("ident", [P, P], CDT, kind="ExternalInput").ap()
    out_ap = nc.dram_tensor("out", [M_LOC, 2 * D], F32, kind="ExternalOutput").ap()

    def load(dst, src):
        if load_engine == "sync":
            nc.sync.dma_start(dst, src)
        else:
            nc.gpsimd.dma_start(dst, src)  # casts f32 -> bf16 inline

    with tile.TileContext(nc) as tc:
        with (
            tc.tile_pool(name="const", bufs=1) as const_pool,
            tc.tile_pool(name="xp", bufs=1) as x_pool,
            tc.tile_pool(name="ap", bufs=2 * N_CHUNKS) as a_pool,
            tc.tile_pool(name="atp", bufs=3) as at_pool,
            tc.tile_pool(name="op", bufs=2) as o_pool,
            tc.tile_pool(name="pt", bufs=2, space="PSUM") as pt_pool,
            tc.tile_pool(name="pacc", bufs=2, space="PSUM") as acc_pool,
        ):
            ident = const_pool.tile([P, P], CDT)
            nc.sync.dma_start(ident[:], id_ap[:])

            x_t = x_pool.tile([P, KB, D], CDT)
            load(x_t[:], x_ap.rearrange("(j p) d -> p j d", p=P))

            for mb in range(MB):
                out_t = o_pool.tile([P, 2 * D], F32)
                for mat, src_ap in ((0, a_ap), (1, a2_ap)):
                    # stream this unit's A rows in 4 chunks
                    chunks = []
                    for c in range(N_CHUNKS):
                        ch = a_pool.tile([P, KB_PER_CHUNK, P], CDT, tag="achunk")
                        sl = src_ap[
                            mb * P : (mb + 1) * P,
                            c * KB_PER_CHUNK * P : (c + 1) * KB_PER_CHUNK * P,
                        ]
                        load(ch[:], sl.rearrange("p (j k) -> p j k", k=P))
                        chunks.append(ch)

                    acc = acc_pool.tile([P, D], F32)

                    def t_group(g):
                        pt = pt_pool.tile([P, GRP, P], CDT, tag="pt")
                        for s in range(GRP):
                            j = g * GRP + s
                            ch = chunks[j // KB_PER_CHUNK]
                            jj = j % KB_PER_CHUNK
                            nc.tensor.transpose(pt[:, s, :], ch[:, jj, :], ident[:])
                        at = at_pool.tile([P, GRP, P], CDT, tag="at")
                        if g % 2 == 0:
                            nc.vector.tensor_copy(at[:], pt[:])
                        else:
                            nc.scalar.copy(at[:], pt[:])
                        return at

                    def mm_group(g, at):
                        for s in range(GRP):
                            j = g * GRP + s
                            nc.tensor.matmul(
                                acc[:],
                                at[:, s, :],
                                x_t[:, j, :],
                                start=(j == 0),
                                stop=(j == KB - 1),
                            )

                    # software pipeline: transpose group g+1 overlaps matmul group g
                    at_prev = t_group(0)
                    for g in range(1, N_GRP):
                        at_next = t_group(g)
                        mm_group(g - 1, at_prev)
                        at_prev = at_next
                    mm_group(N_GRP - 1, at_prev)

                    nc.vector.tensor_copy(out_t[:, mat * D : (mat + 1) * D], acc[:])
                nc.sync.dma_start(out_ap[mb * P : (mb + 1) * P, :], out_t[:])

    nc.compile()
    return nc


def _get_nc(mode):
    if mode not in _cache:
        _cache[mode] = _build(mode)
    return _cache[mode]


def _identity_np(mode):
    if mode == "bf16":
        import ml_dtypes

        return np.eye(P, dtype=ml_dtypes.bfloat16)
    return np.eye(P, dtype=np.float32)


def make_in_maps(x, adj_t, adj_t2, mode=MODE):
    x = np.ascontiguousarray(np.asarray(x, dtype=np.float32))
    adj_t = np.asarray(adj_t, dtype=np.float32)
    adj_t2 = np.asarray(adj_t2, dtype=np.float32)
    ident = _identity_np(mode)
    return [
        {
            "a": adj_t[c * M_LOC : (c + 1) * M_LOC],
            "a2": adj_t2[c * M_LOC : (c + 1) * M_LOC],
            "x": x,
            "ident": ident,
        }
        for c in range(CORES)
    ]


def kernel(x, adj_t, adj_t2):
    from concourse.bass_utils import run_bass_kernel_spmd

    nc = _get_nc(MODE)
    in_maps = make_in_maps(x, adj_t, adj_t2, MODE)
    res = run_bass_kernel_spmd(nc, in_maps, core_ids=list(range(CORES)))
    return np.concatenate([r["out"] for r in res.results], axis=0)


# revision 4
# speedup vs baseline: 196.9480x; 196.9480x over previous
"""H2GCN neighborhood aggregation on 8 Trainium2 NeuronCores.

Computes concat([adj_t @ x, adj_t2 @ x], axis=1) for
adj_t/adj_t2: [8192, 8192] f32, x: [8192, 256] f32.

Sharding: row-shard adj_t/adj_t2 (1024 rows per core), replicate x,
each core produces its [1024, 512] slice of the output.

Per-core dataflow (memory-bound, ~74 MB HBM traffic per core):
  - x loaded once to SBUF (optionally cast f32->bf16 in the DMA).
  - A-row-block tiles streamed in naturally ([128, 2048] chunks),
    PE-transposed 128x128 blocks -> PSUM -> copied to SBUF (DVE/ACT
    alternating), then used as the stationary operand of bf16/f32r
    matmuls accumulating over k into PSUM [128, 256].
"""

import numpy as np

N = 8192
D = 256
CORES = 8
P = 128
M_LOC = N // CORES  # 1024 rows of each adjacency matrix per core
MB = M_LOC // P  # 8 output row-blocks per core
KB = N // P  # 64 contraction blocks
KB_PER_CHUNK = 16  # A streamed in [128, 2048] chunks
N_CHUNKS = KB // KB_PER_CHUNK  # 4
GRP = 8  # k-blocks per transpose/copy group (one PSUM bank)
N_GRP = KB // GRP  # 8

MODE = "bf16"  # "bf16" or "f32r"

_cache = {}


def _build(mode, repeat=1):
    import concourse.bacc as bacc
    import concourse.tile as tile
    import concourse.mybir as mybir

    F32 = mybir.dt.float32
    if mode == "bf16":
        CDT = mybir.dt.bfloat16
        in_dt = F32
        load_engine = None  # gpsimd (cast in DMA)
    else:
        CDT = mybir.dt.float32r
        in_dt = mybir.dt.float32r
        load_engine = "sync"

    nc = bacc.Bacc(
        "TRN2",
        target_bir_lowering=False,
        debug=False,
        enable_asserts=False,
        num_devices=CORES,
    )
    a_ap = nc.dram_tensor("a", [M_LOC, N], in_dt, kind="ExternalInput").ap()
    a2_ap = nc.dram_tensor("a2", [M_LOC, N], in_dt, kind="ExternalInput").ap()
    x_ap = nc.dram_tensor("x", [N, D], in_dt, kind="ExternalInput").ap()
    id_ap = nc.dram_tensor("ident", [P, P], CDT, kind="ExternalInput").ap()
    out_ap = nc.dram_tensor("out", [M_LOC, 2 * D], F32, kind="ExternalOutput").ap()

    def load(dst, src):
        if load_engine == "sync":
            nc.sync.dma_start(dst, src)
        else:
            nc.gpsimd.dma_start(dst, src)  # casts f32 -> bf16 inline

    with tile.TileContext(nc) as tc:
        with (
            tc.tile_pool(name="const", bufs=1) as const_pool,
            tc.tile_pool(name="xp", bufs=1) as x_pool,
            tc.tile_pool(name="ap", bufs=2 * N_CHUNKS) as a_pool,
            tc.tile_pool(name="atp", bufs=3) as at_pool,
            tc.tile_pool(name="op", bufs=2) as o_pool,
            tc.tile_pool(name="pt", bufs=2, space="PSUM") as pt_pool,
            tc.tile_pool(name="pacc", bufs=2, space="PSUM") as acc_pool,
        ):
            ident = const_pool.tile([P, P], CDT)
            nc.sync.dma_start(ident[:], id_ap[:])

            x_t = x_pool.tile([P, KB, D], CDT)
            load(x_t[:], x_ap.rearrange("(j p) d -> p j d", p=P))

            for _rep in range(repeat):
              for mb in range(MB):
                out_t = o_pool.tile([P, 2 * D], F32)
                for mat, src_ap in ((0, a_ap), (1, a2_ap)):
                    # stream this unit's A rows in 4 chunks
                    chunks = []
                    for c in range(N_CHUNKS):
                        ch = a_pool.tile([P, KB_PER_CHUNK, P], CDT, tag="achunk")
                        sl = src_ap[
                            mb * P : (mb + 1) * P,
                            c * KB_PER_CHUNK * P : (c + 1) * KB_PER_CHUNK * P,
                        ]
                        load(ch[:], sl.rearrange("p (j k) -> p j k", k=P))
                        chunks.append(ch)

                    acc = acc_pool.tile([P, D], F32)

                    def t_group(g):
                        pt = pt_pool.tile([P, GRP, P], CDT, tag="pt")
                        for s in range(GRP):
                            j = g * GRP + s
                            ch = chunks[j // KB_PER_CHUNK]
                            jj = j % KB_PER_CHUNK
                            nc.tensor.transpose(pt[:, s, :], ch[:, jj, :], ident[:])
                        at = at_pool.tile([P, GRP, P], CDT, tag="at")
                        if g % 2 == 0:
                            nc.vector.tensor_copy(at[:], pt[:])
                        else:
                            nc.scalar.copy(at[:], pt[:])
                        return at

                    def mm_group(g, at):
                        for s in range(GRP):
                            j = g * GRP + s
                            nc.tensor.matmul(
                                acc[:],
                                at[:, s, :],
                                x_t[:, j, :],
                                start=(j == 0),
                                stop=(j == KB - 1),
                            )

                    # software pipeline: transpose group g+1 overlaps matmul group g
                    at_prev = t_group(0)
                    for g in range(1, N_GRP):
                        at_next = t_group(g)
                        mm_group(g - 1, at_prev)
                        at_prev = at_next
                    mm_group(N_GRP - 1, at_prev)

                    nc.vector.tensor_copy(out_t[:, mat * D : (mat + 1) * D], acc[:])
                nc.sync.dma_start(out_ap[mb * P : (mb + 1) * P, :], out_t[:])

    nc.compile()
    return nc


def _get_nc(mode, repeat=1):
    key = (mode, repeat)
    if key not in _cache:
        _cache[key] = _build(mode, repeat)
    return _cache[key]


def _identity_np(mode):
    if mode == "bf16":
        import ml_dtypes

        return np.eye(P, dtype=ml_dtypes.bfloat16)
    return np.eye(P, dtype=np.float32)


def make_in_maps(x, adj_t, adj_t2, mode=MODE):
    x = np.ascontiguousarray(np.asarray(x, dtype=np.float32))
    adj_t = np.asarray(adj_t, dtype=np.float32)
    adj_t2 = np.asarray(adj_t2, dtype=np.float32)
    ident = _identity_np(mode)
    return [
        {
            "a": adj_t[c * M_LOC : (c + 1) * M_LOC],
            "a2": adj_t2[c * M_LOC : (c + 1) * M_LOC],
            "x": x,
            "ident": ident,
        }
        for c in range(CORES)
    ]


def kernel(x, adj_t, adj_t2):
    from concourse.bass_utils import run_bass_kernel_spmd

    nc = _get_nc(MODE)
    in_maps = make_in_maps(x, adj_t, adj_t2, MODE)
    res = run_bass_kernel_spmd(nc, in_maps, core_ids=list(range(CORES)))
    return np.concatenate([r["out"] for r in res.results], axis=0)


# revision 6
# speedup vs baseline: 342.6383x; 1.7397x over previous
"""H2GCN neighborhood aggregation on 8 Trainium2 NeuronCores.

Computes concat([adj_t @ x, adj_t2 @ x], axis=1) for
adj_t/adj_t2: [8192, 8192] f32, x: [8192, 256] f32.

Sharding: row-shard adj_t/adj_t2 (1024 rows per core), replicate x,
each core produces its [1024, 512] slice of the output.

Per-core dataflow (memory-bound, ~74 MB HBM traffic per core):
  - x loaded once to SBUF (optionally cast f32->bf16 in the DMA).
  - A-row-block tiles streamed in naturally ([128, 2048] chunks),
    PE-transposed 128x128 blocks -> PSUM -> copied to SBUF (DVE/ACT
    alternating), then used as the stationary operand of bf16/f32r
    matmuls accumulating over k into PSUM [128, 256].
"""

import numpy as np

N = 8192
D = 256
CORES = 8
P = 128
M_LOC = N // CORES  # 1024 rows of each adjacency matrix per core
MB = M_LOC // P  # 8 output row-blocks per core
KB = N // P  # 64 contraction blocks
KB_PER_CHUNK = 32  # A streamed in [128, 4096] chunks (2 MB DRAM-side)
N_CHUNKS = KB // KB_PER_CHUNK  # 2
GRP = 8  # k-blocks per transpose/copy group (one PSUM bank)
N_GRP = KB // GRP  # 8

MODE = "bf16"  # "bf16" or "f32r"

_cache = {}


def _build(mode, repeat=1):
    import concourse.bacc as bacc
    import concourse.tile as tile
    import concourse.mybir as mybir

    F32 = mybir.dt.float32
    if mode == "bf16":
        CDT = mybir.dt.bfloat16
        in_dt = F32
        load_engine = None  # gpsimd (cast in DMA)
    else:
        CDT = mybir.dt.float32r
        in_dt = mybir.dt.float32r
        load_engine = "sync"

    nc = bacc.Bacc(
        "TRN2",
        target_bir_lowering=False,
        debug=False,
        enable_asserts=False,
        num_devices=CORES,
    )
    a_ap = nc.dram_tensor("a", [M_LOC, N], in_dt, kind="ExternalInput").ap()
    a2_ap = nc.dram_tensor("a2", [M_LOC, N], in_dt, kind="ExternalInput").ap()
    x_ap = nc.dram_tensor("x", [N, D], in_dt, kind="ExternalInput").ap()
    id_ap = nc.dram_tensor("ident", [P, P], CDT, kind="ExternalInput").ap()
    out_ap = nc.dram_tensor("out", [M_LOC, 2 * D], F32, kind="ExternalOutput").ap()

    def load(dst, src):
        if load_engine == "sync":
            nc.sync.dma_start(dst, src)
        else:
            nc.gpsimd.dma_start(dst, src)  # casts f32 -> bf16 inline

    a_bufs = 6 if mode == "bf16" else 4  # prefetch depth in A-chunk slots
    with tile.TileContext(nc) as tc:
        with (
            tc.tile_pool(name="const", bufs=1) as const_pool,
            tc.tile_pool(name="xp", bufs=1) as x_pool,
            tc.tile_pool(name="ap", bufs=a_bufs) as a_pool,
            tc.tile_pool(name="atp", bufs=4) as at_pool,
            tc.tile_pool(name="op", bufs=2) as o_pool,
            tc.tile_pool(name="pt", bufs=3, space="PSUM") as pt_pool,
            tc.tile_pool(name="pacc", bufs=2, space="PSUM") as acc_pool,
        ):
            ident = const_pool.tile([P, P], CDT)
            nc.sync.dma_start(ident[:], id_ap[:])

            # x loaded once, in per-group chunks so the first matmul group
            # only waits for its own 1 MB slice.
            x_t = x_pool.tile([P, KB, D], CDT)
            x_re = x_ap.rearrange("(j p) d -> p j d", p=P)
            for g in range(N_GRP):
                load(x_t[:, g * GRP : (g + 1) * GRP, :], x_re[:, g * GRP : (g + 1) * GRP, :])

            for _rep in range(repeat):
              for mb in range(MB):
                out_t = o_pool.tile([P, 2 * D], F32)
                for mat, src_ap in ((0, a_ap), (1, a2_ap)):
                    # stream this unit's A rows in 4 chunks
                    chunks = []
                    for c in range(N_CHUNKS):
                        ch = a_pool.tile([P, KB_PER_CHUNK, P], CDT, tag="achunk")
                        sl = src_ap[
                            mb * P : (mb + 1) * P,
                            c * KB_PER_CHUNK * P : (c + 1) * KB_PER_CHUNK * P,
                        ]
                        load(ch[:], sl.rearrange("p (j k) -> p j k", k=P))
                        chunks.append(ch)

                    acc = acc_pool.tile([P, D], F32)

                    def t_group(g):
                        pt = pt_pool.tile([P, GRP, P], CDT, tag="pt")
                        for s in range(GRP):
                            j = g * GRP + s
                            ch = chunks[j // KB_PER_CHUNK]
                            jj = j % KB_PER_CHUNK
                            nc.tensor.transpose(pt[:, s, :], ch[:, jj, :], ident[:])
                        at = at_pool.tile([P, GRP, P], CDT, tag="at")
                        if g % 2 == 0:
                            nc.vector.tensor_copy(at[:], pt[:])
                        else:
                            nc.scalar.copy(at[:], pt[:])
                        return at

                    def mm_group(g, at):
                        for s in range(GRP):
                            j = g * GRP + s
                            nc.tensor.matmul(
                                acc[:],
                                at[:, s, :],
                                x_t[:, j, :],
                                start=(j == 0),
                                stop=(j == KB - 1),
                            )

                    # software pipeline: transpose group g+1 overlaps matmul group g
                    at_prev = t_group(0)
                    for g in range(1, N_GRP):
                        at_next = t_group(g)
                        mm_group(g - 1, at_prev)
                        at_prev = at_next
                    mm_group(N_GRP - 1, at_prev)

                    nc.vector.tensor_copy(out_t[:, mat * D : (mat + 1) * D], acc[:])
                nc.sync.dma_start(out_ap[mb * P : (mb + 1) * P, :], out_t[:])

    nc.compile()
    return nc


def _get_nc(mode, repeat=1):
    key = (mode, repeat)
    if key not in _cache:
        _cache[key] = _build(mode, repeat)
    return _cache[key]


def _identity_np(mode):
    if mode == "bf16":
        import ml_dtypes

        return np.eye(P, dtype=ml_dtypes.bfloat16)
    return np.eye(P, dtype=np.float32)


def make_in_maps(x, adj_t, adj_t2, mode=MODE):
    x = np.ascontiguousarray(np.asarray(x, dtype=np.float32))
    adj_t = np.asarray(adj_t, dtype=np.float32)
    adj_t2 = np.asarray(adj_t2, dtype=np.float32)
    ident = _identity_np(mode)
    return [
        {
            "a": adj_t[c * M_LOC : (c + 1) * M_LOC],
            "a2": adj_t2[c * M_LOC : (c + 1) * M_LOC],
            "x": x,
            "ident": ident,
        }
        for c in range(CORES)
    ]


def kernel(x, adj_t, adj_t2):
    from concourse.bass_utils import run_bass_kernel_spmd

    nc = _get_nc(MODE)
    in_maps = make_in_maps(x, adj_t, adj_t2, MODE)
    res = run_bass_kernel_spmd(nc, in_maps, core_ids=list(range(CORES)))
    return np.concatenate([r["out"] for r in res.results], axis=0)


# revision 10
# speedup vs baseline: 562.4293x; 1.6415x over previous
"""H2GCN neighborhood aggregation on 8 Trainium2 NeuronCores.

Computes concat([adj_t @ x, adj_t2 @ x], axis=1) for
adj_t/adj_t2: [8192, 8192] f32, x: [8192, 256] f32.

Sharding: row-shard adj_t/adj_t2 (1024 rows per core), replicate x,
each core produces its [1024, 512] slice of the output.

Per-core dataflow (memory-bound, ~74 MB HBM traffic per core):
  - x loaded once to SBUF (optionally cast f32->bf16 in the DMA).
  - A-row-block tiles streamed in naturally ([128, 2048] chunks),
    PE-transposed 128x128 blocks -> PSUM -> copied to SBUF (DVE/ACT
    alternating), then used as the stationary operand of bf16/f32r
    matmuls accumulating over k into PSUM [128, 256].
"""

import numpy as np

N = 8192
D = 256
CORES = 8
P = 128
M_LOC = N // CORES  # 1024 rows of each adjacency matrix per core
MB = M_LOC // P  # 8 output row-blocks per core
KB = N // P  # 64 contraction blocks
GRP = 8  # k-blocks per transpose/copy group (one PSUM bank)
N_GRP = KB // GRP  # 8

MODE = "bf16"  # "bf16" or "f32r"

# tuning knobs (referenced at build time; cache key includes them)
TUNE = dict(
    chunk_kb=16,  # k-blocks per A-chunk DMA (16 -> 1 MB DRAM-side)
    x_chunked=True,  # split the x load per matmul group
    a_bufs=8,  # A-chunk slots in flight (2 units of prefetch)
    at_bufs=4,
    pt_bufs=3,
    acc_bufs=2,
)

_cache = {}


def _build(mode, repeat=1):
    KB_PER_CHUNK = TUNE["chunk_kb"]
    N_CHUNKS = KB // KB_PER_CHUNK
    import concourse.bacc as bacc
    import concourse.tile as tile
    import concourse.mybir as mybir

    F32 = mybir.dt.float32
    if mode == "bf16":
        CDT = mybir.dt.bfloat16
        in_dt = F32
        load_engine = None  # gpsimd (cast in DMA)
    else:
        CDT = mybir.dt.float32r
        in_dt = mybir.dt.float32r
        load_engine = "sync"

    nc = bacc.Bacc(
        "TRN2",
        target_bir_lowering=False,
        debug=False,
        enable_asserts=False,
        num_devices=CORES,
    )
    a_ap = nc.dram_tensor("a", [M_LOC, N], in_dt, kind="ExternalInput").ap()
    a2_ap = nc.dram_tensor("a2", [M_LOC, N], in_dt, kind="ExternalInput").ap()
    x_ap = nc.dram_tensor("x", [N, D], in_dt, kind="ExternalInput").ap()
    id_ap = nc.dram_tensor("ident", [P, P], CDT, kind="ExternalInput").ap()
    out_ap = nc.dram_tensor("out", [M_LOC, 2 * D], F32, kind="ExternalOutput").ap()

    def load(dst, src):
        if load_engine == "sync":
            nc.sync.dma_start(dst, src)
        else:
            nc.gpsimd.dma_start(dst, src)  # casts f32 -> bf16 inline

    a_bufs = TUNE["a_bufs"] if mode == "bf16" else 4
    with tile.TileContext(nc) as tc:
        with (
            tc.tile_pool(name="const", bufs=1) as const_pool,
            tc.tile_pool(name="xp", bufs=1) as x_pool,
            tc.tile_pool(name="ap", bufs=a_bufs) as a_pool,
            tc.tile_pool(name="atp", bufs=TUNE["at_bufs"]) as at_pool,
            tc.tile_pool(name="op", bufs=2) as o_pool,
            tc.tile_pool(name="pt", bufs=TUNE["pt_bufs"], space="PSUM") as pt_pool,
            tc.tile_pool(name="pacc", bufs=TUNE["acc_bufs"], space="PSUM") as acc_pool,
        ):
            ident = const_pool.tile([P, P], CDT)
            nc.sync.dma_start(ident[:], id_ap[:])

            # x loaded once; optionally in per-group chunks so the first
            # matmul group only waits for its own 1 MB slice.
            x_t = x_pool.tile([P, KB, D], CDT)
            x_re = x_ap.rearrange("(j p) d -> p j d", p=P)
            if TUNE["x_chunked"]:
                for g in range(N_GRP):
                    load(x_t[:, g * GRP : (g + 1) * GRP, :], x_re[:, g * GRP : (g + 1) * GRP, :])
            else:
                load(x_t[:], x_re)

            for _rep in range(repeat):
              for mb in range(MB):
                out_t = o_pool.tile([P, 2 * D], F32)
                for mat, src_ap in ((0, a_ap), (1, a2_ap)):
                    # stream this unit's A rows in 4 chunks
                    chunks = []
                    for c in range(N_CHUNKS):
                        ch = a_pool.tile([P, KB_PER_CHUNK, P], CDT, tag="achunk")
                        sl = src_ap[
                            mb * P : (mb + 1) * P,
                            c * KB_PER_CHUNK * P : (c + 1) * KB_PER_CHUNK * P,
                        ]
                        load(ch[:], sl.rearrange("p (j k) -> p j k", k=P))
                        chunks.append(ch)

                    acc = acc_pool.tile([P, D], F32)

                    def t_group(g):
                        pt = pt_pool.tile([P, GRP, P], CDT, tag="pt")
                        for s in range(GRP):
                            j = g * GRP + s
                            ch = chunks[j // KB_PER_CHUNK]
                            jj = j % KB_PER_CHUNK
                            nc.tensor.transpose(pt[:, s, :], ch[:, jj, :], ident[:])
                        at = at_pool.tile([P, GRP, P], CDT, tag="at")
                        if g % 2 == 0:
                            nc.vector.tensor_copy(at[:], pt[:])
                        else:
                            nc.scalar.copy(at[:], pt[:])
                        return at

                    def mm_group(g, at):
                        for s in range(GRP):
                            j = g * GRP + s
                            nc.tensor.matmul(
                                acc[:],
                                at[:, s, :],
                                x_t[:, j, :],
                                start=(j == 0),
                                stop=(j == KB - 1),
                            )

                    # software pipeline: transpose group g+1 overlaps matmul group g
                    at_prev = t_group(0)
                    for g in range(1, N_GRP):
                        at_next = t_group(g)
                        mm_group(g - 1, at_prev)
                        at_prev = at_next
                    mm_group(N_GRP - 1, at_prev)

                    nc.vector.tensor_copy(out_t[:, mat * D : (mat + 1) * D], acc[:])
                nc.sync.dma_start(out_ap[mb * P : (mb + 1) * P, :], out_t[:])

    nc.compile()
    return nc


def _get_nc(mode, repeat=1):
    key = (mode, repeat, tuple(sorted(TUNE.items())))
    if key not in _cache:
        _cache[key] = _build(mode, repeat)
    return _cache[key]


def _identity_np(mode):
    if mode == "bf16":
        import ml_dtypes

        return np.eye(P, dtype=ml_dtypes.bfloat16)
    return np.eye(P, dtype=np.float32)


def make_in_maps(x, adj_t, adj_t2, mode=MODE):
    x = np.ascontiguousarray(np.asarray(x, dtype=np.float32))
    adj_t = np.asarray(adj_t, dtype=np.float32)
    adj_t2 = np.asarray(adj_t2, dtype=np.float32)
    ident = _identity_np(mode)
    return [
        {
            "a": adj_t[c * M_LOC : (c + 1) * M_LOC],
            "a2": adj_t2[c * M_LOC : (c + 1) * M_LOC],
            "x": x,
            "ident": ident,
        }
        for c in range(CORES)
    ]


def kernel(x, adj_t, adj_t2):
    from concourse.bass_utils import run_bass_kernel_spmd

    nc = _get_nc(MODE)
    in_maps = make_in_maps(x, adj_t, adj_t2, MODE)
    res = run_bass_kernel_spmd(nc, in_maps, core_ids=list(range(CORES)))
    return np.concatenate([r["out"] for r in res.results], axis=0)


# revision 15
# speedup vs baseline: 861.7244x; 1.5321x over previous
"""H2GCN neighborhood aggregation on 8 Trainium2 NeuronCores.

Computes concat([adj_t @ x, adj_t2 @ x], axis=1) for
adj_t/adj_t2: [8192, 8192] f32, x: [8192, 256] f32.

Sharding: row-shard adj_t/adj_t2 (1024 rows per core), replicate x,
each core produces its [1024, 512] slice of the output.

Per-core dataflow (memory-bound, ~74 MB HBM traffic per core):
  - x loaded once to SBUF (optionally cast f32->bf16 in the DMA).
  - A-row-block tiles streamed in naturally ([128, 2048] chunks),
    PE-transposed 128x128 blocks -> PSUM -> copied to SBUF (DVE/ACT
    alternating), then used as the stationary operand of bf16/f32r
    matmuls accumulating over k into PSUM [128, 256].
"""

import numpy as np

N = 8192
D = 256
CORES = 8
P = 128
M_LOC = N // CORES  # 1024 rows of each adjacency matrix per core
MB = M_LOC // P  # 8 output row-blocks per core
KB = N // P  # 64 contraction blocks
GRP = 8  # k-blocks per transpose/copy group (one PSUM bank)
N_GRP = KB // GRP  # 8

MODE = "bf16t"  # "bf16t" (host-cast + DMA-transpose), "bf16", or "f32r"

# tuning knobs (referenced at build time; cache key includes them)
TUNE = dict(
    chunk_kb=16,  # k-blocks per A-chunk DMA (16 -> 1 MB DRAM-side)
    x_chunked=True,  # split the x load per matmul group
    a_bufs=8,  # A-chunk slots in flight (2 units of prefetch)
    at_bufs=4,
    pt_bufs=3,
    acc_bufs=2,
)

_cache = {}


def _build_bf16t(repeat=1):
    """Inputs pre-cast to bf16 on host (halves adjacency HBM traffic).
    A column-stripes [1024, 128] are loaded via the HW xbar DMA-transpose
    directly into matmul-ready [128k, 1024m] layout — no PE transposes, no
    PSUM round-trip. 8 PSUM banks hold one accumulator per output row-block."""
    import concourse.bacc as bacc
    import concourse.tile as tile
    import concourse.mybir as mybir

    F32 = mybir.dt.float32
    BF16 = mybir.dt.bfloat16

    nc = bacc.Bacc(
        "TRN2",
        target_bir_lowering=False,
        debug=False,
        enable_asserts=False,
        num_devices=CORES,
    )
    a_ap = nc.dram_tensor("a", [M_LOC, N], BF16, kind="ExternalInput").ap()
    a2_ap = nc.dram_tensor("a2", [M_LOC, N], BF16, kind="ExternalInput").ap()
    x_ap = nc.dram_tensor("x", [N, D], BF16, kind="ExternalInput").ap()
    out_ap = nc.dram_tensor("out", [M_LOC, 2 * D], F32, kind="ExternalOutput").ap()

    with tile.TileContext(nc) as tc:
        with (
            tc.tile_pool(name="xp", bufs=1) as x_pool,
            tc.tile_pool(name="stp", bufs=6) as st_pool,
            tc.tile_pool(name="op", bufs=MB) as o_pool,
            tc.tile_pool(name="pacc", bufs=MB, space="PSUM") as acc_pool,
        ):
            # x first, via plain DMA, before any xbar-transpose DMA (Tile
            # serializes on xbar-mode transitions; keep transitions rare).
            x_t = x_pool.tile([P, KB, D], BF16)
            x_re = x_ap.rearrange("(j p) d -> p j d", p=P)
            for g in range(N_GRP):
                nc.sync.dma_start(
                    x_t[:, g * GRP : (g + 1) * GRP, :],
                    x_re[:, g * GRP : (g + 1) * GRP, :],
                )

            out_ts = [o_pool.tile([P, 2 * D], F32, tag="outt", name=f"outt{i}") for i in range(MB)]
            for _rep in range(repeat):
                for mat, src_ap in ((0, a_ap), (1, a2_ap)):
                    accs = [acc_pool.tile([P, D], F32, tag="acc", name=f"acc{i}") for i in range(MB)]
                    for k in range(KB):
                        st = st_pool.tile([P, M_LOC], BF16, tag="stripe")
                        nc.sync.dma_start_transpose(
                            st[:], src_ap[:, k * P : (k + 1) * P]
                        )
                        for mb in range(MB):
                            nc.tensor.matmul(
                                accs[mb][:],
                                st[:, mb * P : (mb + 1) * P],
                                x_t[:, k, :],
                                start=(k == 0),
                                stop=(k == KB - 1),
                            )
                    for mb in range(MB):
                        if mb % 2 == 0:
                            nc.vector.tensor_copy(
                                out_ts[mb][:, mat * D : (mat + 1) * D], accs[mb][:]
                            )
                        else:
                            nc.scalar.copy(
                                out_ts[mb][:, mat * D : (mat + 1) * D], accs[mb][:]
                            )
                for mb in range(MB):
                    nc.sync.dma_start(out_ap[mb * P : (mb + 1) * P, :], out_ts[mb][:])

    nc.compile()
    return nc


def _build(mode, repeat=1):
    if mode == "bf16t":
        return _build_bf16t(repeat)
    KB_PER_CHUNK = TUNE["chunk_kb"]
    N_CHUNKS = KB // KB_PER_CHUNK
    import concourse.bacc as bacc
    import concourse.tile as tile
    import concourse.mybir as mybir

    F32 = mybir.dt.float32
    if mode == "bf16":
        CDT = mybir.dt.bfloat16
        in_dt = F32
        load_engine = None  # gpsimd (cast in DMA)
    else:
        CDT = mybir.dt.float32r
        in_dt = mybir.dt.float32r
        load_engine = "sync"

    nc = bacc.Bacc(
        "TRN2",
        target_bir_lowering=False,
        debug=False,
        enable_asserts=False,
        num_devices=CORES,
    )
    a_ap = nc.dram_tensor("a", [M_LOC, N], in_dt, kind="ExternalInput").ap()
    a2_ap = nc.dram_tensor("a2", [M_LOC, N], in_dt, kind="ExternalInput").ap()
    x_ap = nc.dram_tensor("x", [N, D], in_dt, kind="ExternalInput").ap()
    id_ap = nc.dram_tensor("ident", [P, P], CDT, kind="ExternalInput").ap()
    out_ap = nc.dram_tensor("out", [M_LOC, 2 * D], F32, kind="ExternalOutput").ap()

    def load(dst, src):
        if load_engine == "sync":
            nc.sync.dma_start(dst, src)
        else:
            nc.gpsimd.dma_start(dst, src)  # casts f32 -> bf16 inline

    a_bufs = TUNE["a_bufs"] if mode == "bf16" else 4
    with tile.TileContext(nc) as tc:
        with (
            tc.tile_pool(name="const", bufs=1) as const_pool,
            tc.tile_pool(name="xp", bufs=1) as x_pool,
            tc.tile_pool(name="ap", bufs=a_bufs) as a_pool,
            tc.tile_pool(name="atp", bufs=TUNE["at_bufs"]) as at_pool,
            tc.tile_pool(name="op", bufs=2) as o_pool,
            tc.tile_pool(name="pt", bufs=TUNE["pt_bufs"], space="PSUM") as pt_pool,
            tc.tile_pool(name="pacc", bufs=TUNE["acc_bufs"], space="PSUM") as acc_pool,
        ):
            ident = const_pool.tile([P, P], CDT)
            nc.sync.dma_start(ident[:], id_ap[:])

            # x loaded once; optionally in per-group chunks so the first
            # matmul group only waits for its own 1 MB slice.
            x_t = x_pool.tile([P, KB, D], CDT)
            x_re = x_ap.rearrange("(j p) d -> p j d", p=P)
            if TUNE["x_chunked"]:
                for g in range(N_GRP):
                    load(x_t[:, g * GRP : (g + 1) * GRP, :], x_re[:, g * GRP : (g + 1) * GRP, :])
            else:
                load(x_t[:], x_re)

            for _rep in range(repeat):
              for mb in range(MB):
                out_t = o_pool.tile([P, 2 * D], F32)
                for mat, src_ap in ((0, a_ap), (1, a2_ap)):
                    # stream this unit's A rows in 4 chunks
                    chunks = []
                    for c in range(N_CHUNKS):
                        ch = a_pool.tile([P, KB_PER_CHUNK, P], CDT, tag="achunk")
                        sl = src_ap[
                            mb * P : (mb + 1) * P,
                            c * KB_PER_CHUNK * P : (c + 1) * KB_PER_CHUNK * P,
                        ]
                        load(ch[:], sl.rearrange("p (j k) -> p j k", k=P))
                        chunks.append(ch)

                    acc = acc_pool.tile([P, D], F32)

                    def t_group(g):
                        pt = pt_pool.tile([P, GRP, P], CDT, tag="pt")
                        for s in range(GRP):
                            j = g * GRP + s
                            ch = chunks[j // KB_PER_CHUNK]
                            jj = j % KB_PER_CHUNK
                            nc.tensor.transpose(pt[:, s, :], ch[:, jj, :], ident[:])
                        at = at_pool.tile([P, GRP, P], CDT, tag="at")
                        if g % 2 == 0:
                            nc.vector.tensor_copy(at[:], pt[:])
                        else:
                            nc.scalar.copy(at[:], pt[:])
                        return at

                    def mm_group(g, at):
                        for s in range(GRP):
                            j = g * GRP + s
                            nc.tensor.matmul(
                                acc[:],
                                at[:, s, :],
                                x_t[:, j, :],
                                start=(j == 0),
                                stop=(j == KB - 1),
                            )

                    # software pipeline: transpose group g+1 overlaps matmul group g
                    at_prev = t_group(0)
                    for g in range(1, N_GRP):
                        at_next = t_group(g)
                        mm_group(g - 1, at_prev)
                        at_prev = at_next
                    mm_group(N_GRP - 1, at_prev)

                    nc.vector.tensor_copy(out_t[:, mat * D : (mat + 1) * D], acc[:])
                nc.sync.dma_start(out_ap[mb * P : (mb + 1) * P, :], out_t[:])

    nc.compile()
    return nc


def _get_nc(mode, repeat=1):
    key = (mode, repeat, tuple(sorted(TUNE.items())))
    if key not in _cache:
        _cache[key] = _build(mode, repeat)
    return _cache[key]


def _identity_np(mode):
    if mode == "bf16":
        import ml_dtypes

        return np.eye(P, dtype=ml_dtypes.bfloat16)
    return np.eye(P, dtype=np.float32)


def make_in_maps(x, adj_t, adj_t2, mode=MODE):
    x = np.ascontiguousarray(np.asarray(x, dtype=np.float32))
    adj_t = np.asarray(adj_t, dtype=np.float32)
    adj_t2 = np.asarray(adj_t2, dtype=np.float32)
    if mode == "bf16t":
        import ml_dtypes

        bf = ml_dtypes.bfloat16
        x = x.astype(bf)
        adj_t = adj_t.astype(bf)
        adj_t2 = adj_t2.astype(bf)
        return [
            {
                "a": adj_t[c * M_LOC : (c + 1) * M_LOC],
                "a2": adj_t2[c * M_LOC : (c + 1) * M_LOC],
                "x": x,
            }
            for c in range(CORES)
        ]
    ident = _identity_np(mode)
    return [
        {
            "a": adj_t[c * M_LOC : (c + 1) * M_LOC],
            "a2": adj_t2[c * M_LOC : (c + 1) * M_LOC],
            "x": x,
            "ident": ident,
        }
        for c in range(CORES)
    ]


def kernel(x, adj_t, adj_t2):
    from concourse.bass_utils import run_bass_kernel_spmd

    nc = _get_nc(MODE)
    in_maps = make_in_maps(x, adj_t, adj_t2, MODE)
    res = run_bass_kernel_spmd(nc, in_maps, core_ids=list(range(CORES)))
    return np.concatenate([r["out"] for r in res.results], axis=0)
